# revision 1
# baseline (speedup 1.0000x reference)
"""Trainium2 Bass kernel for a dense transformer block (nn_Block_30262339567972).

Full inputs in, full outputs out. Internally sharded across 8 NeuronCores with
zero collectives: core c = 2*b + j owns two 512-token chunks of batch b
(j=0 -> chunks {0,3}, j=1 -> chunks {1,2}; the pairing balances causal
attention work). Each core computes LN1 and K/V for the whole 2048-token
sequence itself, Q/attention/proj/MLP only for its own 1024 tokens, and
writes its tokens' final output. The host concatenates.

v2 design notes (vs the f32r baseline):
- All matmul operands are bf16 (PSUM accumulation stays f32). This halves
  LDWEIGHTS time, halves DMA/SBUF, and allows 1024-wide moving operands.
  The residual signal path (x + attn + mlp) stays f32.
- K/V/Q stay SBUF-resident between projection and attention (no DRAM
  round-trip).
- Causal masking is multiplicative {0,1} AFTER exp (scores can't overflow
  exp), so the mask work runs on cheap bf16 2x-mode DVE ops and PSUM is
  touched only by ACT.
- fc2 accumulates over all of d_ff in PSUM (full h1 kept in SBUF as bf16),
  eliminating the per-chunk DVE accumulation.
- Row reciprocals use reciprocal_approx_fast (~5x faster than
  vector.reciprocal; softmax denominators and LN rstd are far from its
  edge cases).
"""

from contextlib import ExitStack

import numpy as np
import ml_dtypes

import concourse.bacc as bacc
import concourse.bass as bass
import concourse.tile as tile
from concourse import mybir
from concourse.bass_utils import run_bass_kernel_spmd
import concourse.bass_utils as _bu


DEBUG = False
F32 = mybir.dt.float32
F32R = mybir.dt.float32r
BF16 = mybir.dt.bfloat16
P = 128
B, T, C = 4, 2048, 1024
H, D = 16, 64
DFF = 4096
TOWN = 1024            # tokens owned per core
EPS = 1e-5
SCALE = D ** -0.5

KT_C = C // P          # 8 contraction tiles over C
FT_C = C // P          # 8 feature tiles over C
TT_FULL = T // P       # 16 token tiles (full seq)
TT_OWN = TOWN // P     # 8 token tiles (own)
NGROUP = H // 2        # 8 head-pair groups

Ident = mybir.ActivationFunctionType.Identity
Sqrt = mybir.ActivationFunctionType.Sqrt
Exp = mybir.ActivationFunctionType.Exp
Relu = mybir.ActivationFunctionType.Relu
ADD = mybir.AluOpType.add
MULT = mybir.AluOpType.mult


def _alloc(pool, n, shape, dt, prefix, **kw):
    return [
        pool.tile(list(shape), dt, tag=f"{prefix}{i}", name=f"{prefix}{i}", **kw)
        for i in range(n)
    ]


def _ln_b16(nc, x_loader, nblocks, dst, dst_off, g_col, b_col, eps_t, ones_b,
            st_ps, rowp, bcp, apl, prefix):
    """LayerNorm, feature-major, bf16 in/out.

    x_loader(kt, nb) -> [P, 512] bf16 AP. dst: FT_C tiles (bf16); block nb's
    output goes to dst[ft][:, dst_off + nb*512 : ...]. Stats via ones-matmul
    partition reductions in f32 PSUM; mean/rstd rows cast to bf16 and
    partition-broadcast; apply = 2 bf16 DVE ops + ACT gamma/beta."""
    for nb in range(nblocks):
        sl = slice(dst_off + nb * 512, dst_off + (nb + 1) * 512)
        xb = [x_loader(kt, nb) for kt in range(KT_C)]
        ssum = st_ps.tile([P, 512], F32, tag="ss", name=f"{prefix}ss{nb}")
        ssq = st_ps.tile([P, 512], F32, tag="sq", name=f"{prefix}sq{nb}")
        for kt in range(KT_C):
            nc.tensor.matmul(ssum, ones_b, xb[kt],
                             start=(kt == 0), stop=(kt == KT_C - 1))
        for kt in range(KT_C):
            sq = rowp.tile([P, 512], BF16, tag=f"sqt{kt}",
                           name=f"{prefix}sqt{nb}_{kt}", bufs=1)
            nc.vector.tensor_mul(out=sq, in0=xb[kt], in1=xb[kt])
            nc.tensor.matmul(ssq, ones_b, sq,
                             start=(kt == 0), stop=(kt == KT_C - 1))
        mu = rowp.tile([1, 512], F32, tag="mu", name=f"{prefix}mu{nb}")
        nc.scalar.mul(mu, ssum[0:1, :], 1.0 / C)
        msq = rowp.tile([1, 512], F32, tag="msq", name=f"{prefix}msq{nb}")
        nc.scalar.mul(msq, ssq[0:1, :], 1.0 / C)
        var = rowp.tile([1, 512], F32, tag="mu2", name=f"{prefix}var{nb}")
        nc.vector.tensor_mul(out=var, in0=mu, in1=mu)
        nc.vector.tensor_sub(out=var, in0=msq, in1=var)
        std = rowp.tile([1, 512], F32, tag="std", name=f"{prefix}std{nb}")
        nc.scalar.activation(out=std, in_=var, func=Sqrt,
                             bias=eps_t[0:1, 0:1], scale=1.0)
        rs = rowp.tile([1, 512], F32, tag="rs", name=f"{prefix}rs{nb}")
        nc.vector.reciprocal_approx_fast(out=rs, in_=std)
        mu16 = rowp.tile([1, 512], BF16, tag="mu16", name=f"{prefix}mu16{nb}")
        nc.vector.tensor_copy(out=mu16, in_=mu)
        rs16 = rowp.tile([1, 512], BF16, tag="rs16", name=f"{prefix}rs16{nb}")
        nc.vector.tensor_copy(out=rs16, in_=rs)
        mu_b = bcp.tile([P, 512], BF16, tag="mub", name=f"{prefix}mub{nb}")
        nc.gpsimd.partition_broadcast(mu_b, mu16)
        rs_b = bcp.tile([P, 512], BF16, tag="rsb", name=f"{prefix}rsb{nb}")
        nc.gpsimd.partition_broadcast(rs_b, rs16)
        for ft in range(FT_C):
            t = apl.tile([P, 512], BF16, tag=f"ap{ft}",
                         name=f"{prefix}ap{nb}_{ft}", bufs=1)
            nc.vector.tensor_sub(out=t, in0=xb[ft], in1=mu_b)
            nc.vector.tensor_mul(out=t, in0=t, in1=rs_b)
            nc.scalar.activation(out=dst[ft][:, sl], in_=t, func=Ident,
                                 bias=b_col[:, ft:ft + 1],
                                 scale=g_col[:, ft:ft + 1])


def build_nc():
    nc = bacc.Bacc()
    xT_b16 = nc.declare_dram_parameter("xT_b16", [C, T], BF16, isOutput=False)
    xTo_b16 = nc.declare_dram_parameter("xTo_b16", [C, TOWN], BF16, isOutput=False)
    xTo_f32 = nc.declare_dram_parameter("xTo_f32", [C, TOWN], F32, isOutput=False)
    mask0 = nc.declare_dram_parameter("mask0", [512, 1024], BF16, isOutput=False)
    mask1 = nc.declare_dram_parameter("mask1", [512, 1024], BF16, isOutput=False)
    attn_w = nc.declare_dram_parameter("attn_w", [C, 3 * C], BF16, isOutput=False)
    attn_b = nc.declare_dram_parameter("attn_b", [3 * C], F32, isOutput=False)
    proj_w = nc.declare_dram_parameter("proj_w", [C, C], BF16, isOutput=False)
    proj_b = nc.declare_dram_parameter("proj_b", [C], F32, isOutput=False)
    ln1_g = nc.declare_dram_parameter("ln1_g", [C], F32, isOutput=False)
    ln1_b = nc.declare_dram_parameter("ln1_b", [C], F32, isOutput=False)
    ln2_g = nc.declare_dram_parameter("ln2_g", [C], F32, isOutput=False)
    ln2_b = nc.declare_dram_parameter("ln2_b", [C], F32, isOutput=False)
    fc1_w = nc.declare_dram_parameter("fc1_w", [C, DFF], BF16, isOutput=False)
    fc1_b = nc.declare_dram_parameter("fc1_b", [DFF], F32, isOutput=False)
    fc2_w = nc.declare_dram_parameter("fc2_w", [DFF, C], BF16, isOutput=False)
    fc2_b = nc.declare_dram_parameter("fc2_b", [C], F32, isOutput=False)
    out = nc.declare_dram_parameter("out", [C, TOWN], F32, isOutput=True)
    if DEBUG:
        dbg_ho = nc.declare_dram_parameter("dbg_ho", [C, TOWN], BF16, isOutput=True)
        dbg_hf = nc.declare_dram_parameter("dbg_hf", [C, T], BF16, isOutput=True)
        dbg_q = nc.declare_dram_parameter("dbg_q", [C, TOWN], BF16, isOutput=True)
        dbg_k = nc.declare_dram_parameter("dbg_k", [C, T], BF16, isOutput=True)
        dbg_v = nc.declare_dram_parameter("dbg_v", [P, TT_FULL * 16 * 65], BF16, isOutput=True)
        dbg_at = nc.declare_dram_parameter("dbg_at", [C, TOWN], BF16, isOutput=True)
        dbg_x2 = nc.declare_dram_parameter("dbg_x2", [C, TOWN], F32, isOutput=True)
        dbg_h2 = nc.declare_dram_parameter("dbg_h2", [C, TOWN], BF16, isOutput=True)
        dbg_r = nc.declare_dram_parameter("dbg_r", [32, 512], F32, isOutput=True)
        dbg_pt = nc.declare_dram_parameter("dbg_pt", [16 * P, 1024], BF16, isOutput=True)

    with tile.TileContext(nc) as tc, ExitStack() as top:
        const = top.enter_context(tc.tile_pool(name="const", bufs=1))
        eps_t = const.tile([P, 1], F32, name="eps_t")
        nc.vector.memset(eps_t, EPS)
        ones_f = const.tile([P, 1], F32, name="ones_f")
        nc.vector.memset(ones_f, 1.0)
        ones_b = const.tile([P, P], BF16, name="ones_b")
        nc.vector.memset(ones_b, 1.0)
        ones_r = const.tile([P, 1], F32R, name="ones_r")
        nc.vector.tensor_copy(out=ones_r, in_=ones_f)
        ln1g_t = const.tile([P, FT_C], F32, name="ln1g_t")
        ln1b_t = const.tile([P, FT_C], F32, name="ln1b_t")
        ln2g_t = const.tile([P, FT_C], F32, name="ln2g_t")
        ln2b_t = const.tile([P, FT_C], F32, name="ln2b_t")
        nc.scalar.dma_start(out=ln1g_t, in_=ln1_g.rearrange("(f p) -> p f", p=P))
        nc.scalar.dma_start(out=ln1b_t, in_=ln1_b.rearrange("(f p) -> p f", p=P))
        nc.scalar.dma_start(out=ln2g_t, in_=ln2_g.rearrange("(f p) -> p f", p=P))
        nc.scalar.dma_start(out=ln2b_t, in_=ln2_b.rearrange("(f p) -> p f", p=P))
        abq_t = const.tile([P, NGROUP], F32, name="abq_t")
        abk_t = const.tile([P, NGROUP], F32, name="abk_t")
        nc.sync.dma_start(out=abq_t, in_=attn_b[0:C].rearrange("(g p) -> p g", p=P))
        nc.sync.dma_start(out=abk_t,
                          in_=attn_b[C:2 * C].rearrange("(g p) -> p g", p=P))
        projb_t = const.tile([P, FT_C], F32, name="projb_t")
        nc.scalar.dma_start(out=projb_t, in_=proj_b.rearrange("(f p) -> p f", p=P))
        fc2b_t = const.tile([P, FT_C], F32, name="fc2b_t")
        nc.scalar.dma_start(out=fc2b_t, in_=fc2_b.rearrange("(f p) -> p f", p=P))
        fc1b_t = const.tile([P, DFF // P], F32, name="fc1b_t")
        nc.scalar.dma_start(out=fc1b_t, in_=fc1_b.rearrange("(f p) -> p f", p=P))
        bv_bc = const.tile([P, C], F32, name="bv_bc")
        abv = attn_b[2 * C:3 * C]
        nc.sync.dma_start(
            out=bv_bc,
            in_=bass.AP(tensor=abv.tensor, offset=abv.offset,
                        ap=[[0, P]] + list(abv.ap[-1:])))

        # K/V/Q live from phase A until the end of attention
        s1 = ExitStack()
        qkvp = s1.enter_context(tc.tile_pool(name="qkvp", bufs=1))
        qT = _alloc(qkvp, NGROUP, [P, TOWN], BF16, "qT")
        kT = _alloc(qkvp, NGROUP, [P, T], BF16, "kT")
        vG = qkvp.tile([P, TT_FULL * 2 * NGROUP * 65], BF16, name="vG")
        vG4 = vG.rearrange("p (t h x) -> p t h x", t=TT_FULL, h=2 * NGROUP)

        # ---- Phase A: LN1 + QKV projections ----
        with ExitStack() as cA:
            st_ps = cA.enter_context(
                tc.tile_pool(name="st_ps", bufs=2, space="PSUM"))
            mm_ps = cA.enter_context(
                tc.tile_pool(name="mm_ps", bufs=2, space="PSUM"))
            rowp = cA.enter_context(tc.tile_pool(name="rowp", bufs=1))
            bcp = cA.enter_context(tc.tile_pool(name="bcp", bufs=2))
            apl = cA.enter_context(tc.tile_pool(name="apl", bufs=1))
            hp = cA.enter_context(tc.tile_pool(name="hp", bufs=1))
            wp = cA.enter_context(tc.tile_pool(name="wp", bufs=1))
            lnp = cA.enter_context(tc.tile_pool(name="lnp", bufs=1))

            hTo = _alloc(hp, FT_C, [P, TOWN], BF16, "hTo")
            hTf = _alloc(hp, FT_C, [P, T], BF16, "hTf")

            _dmaq = [nc.sync, nc.scalar, nc.gpsimd]

            def own_loader(kt, nb):
                t = lnp.tile([P, 512], BF16, tag=f"x{kt}",
                             name=f"xo{kt}_{nb}", bufs=1)
                _dmaq[kt % 3].dma_start(
                    out=t, in_=xTo_b16[kt * P:(kt + 1) * P,
                                       nb * 512:(nb + 1) * 512])
                return t[:, :]

            _ln_b16(nc, own_loader, 2, hTo, 0, ln1g_t, ln1b_t, eps_t, ones_b,
                    st_ps, rowp, bcp, apl, "lo")

            for half in range(2):
                def full_loader(kt, nb, _h=half):
                    s = _h * TOWN + nb * 512
                    t = lnp.tile([P, 512], BF16, tag=f"x{kt}",
                                 name=f"xf{_h}_{kt}_{nb}", bufs=1)
                    _dmaq[kt % 3].dma_start(
                        out=t, in_=xT_b16[kt * P:(kt + 1) * P, s:s + 512])
                    return t[:, :]
                _ln_b16(nc, full_loader, 2, hTf, half * TOWN, ln1g_t, ln1b_t,
                        eps_t, ones_b, st_ps, rowp, bcp, apl, f"lf{half}")

            # streamed weight tiles: Q then K then V reuse the same slots
            def wload(col0, idx, nm):
                w = wp.tile([P, C], BF16, tag=f"w{idx}", name=nm, bufs=2)
                _dmaq[idx % 2].dma_start(
                    out=w, in_=attn_w[idx * P:(idx + 1) * P, col0:col0 + C])
                return w

            wq = [wload(0, kt, f"wq{kt}") for kt in range(KT_C)]

            # Q for own tokens
            for g in range(NGROUP):
                qps = [mm_ps.tile([P, 512], F32, tag=f"mm{nb}",
                                  name=f"qps{g}_{nb}") for nb in range(2)]
                for kt in range(KT_C):
                    for nb in range(2):
                        nc.tensor.matmul(
                            qps[nb], wq[kt][:, g * P:(g + 1) * P],
                            hTo[kt][:, nb * 512:(nb + 1) * 512],
                            start=(kt == 0), stop=(kt == KT_C - 1))
                for nb in range(2):
                    nc.vector.tensor_scalar_add(
                        out=qT[g][:, nb * 512:(nb + 1) * 512], in0=qps[nb],
                        scalar1=abq_t[:, g:g + 1])

            # K columns for the full sequence
            wk = [wload(C, kt, f"wk{kt}") for kt in range(KT_C)]
            for half in range(2):
                hsl = slice(half * TOWN, (half + 1) * TOWN)
                for g in range(NGROUP):
                    kps = [mm_ps.tile([P, 512], F32, tag=f"mm{nb}",
                                      name=f"kps{half}_{g}_{nb}")
                           for nb in range(2)]
                    for kt in range(KT_C):
                        for nb in range(2):
                            s = half * TOWN + nb * 512
                            nc.tensor.matmul(
                                kps[nb], wk[kt][:, g * P:(g + 1) * P],
                                hTf[kt][:, s:s + 512],
                                start=(kt == 0), stop=(kt == KT_C - 1))
                    for nb in range(2):
                        s = half * TOWN + nb * 512
                        nc.vector.tensor_scalar_add(
                            out=kT[g][:, s:s + 512], in0=kps[nb],
                            scalar1=abk_t[:, g:g + 1])

            # V rows (token-major) for the full sequence
            nc.gpsimd.memset(vG4[:, :, :, 64:65], 1.0)  # softmax denom ones
            wv = [wload(2 * C, kt, f"wv{kt}") for kt in range(KT_C)]
            for tt in range(TT_FULL):
                vps = [mm_ps.tile([P, 512], F32, tag=f"mm{nb}",
                                  name=f"vps{tt}_{nb}") for nb in range(2)]
                for kt in range(KT_C):
                    for nb in range(2):
                        nc.tensor.matmul(
                            vps[nb], hTf[kt][:, tt * P:(tt + 1) * P],
                            wv[kt][:, nb * 512:(nb + 1) * 512],
                            start=(kt == 0), stop=(kt == KT_C - 1))
                for nb in range(2):
                    nc.vector.tensor_add(
                        out=vG4[:, tt, nb * 8:(nb + 1) * 8, 0:64],
                        in0=vps[nb].rearrange("p (h d) -> p h d", d=64),
                        in1=bv_bc[:, nb * 512:(nb + 1) * 512].rearrange(
                            "p (h d) -> p h d", d=64))

        if DEBUG:
            for i in range(FT_C):
                nc.sync.dma_start(out=dbg_ho[i * P:(i + 1) * P, :], in_=hTo[i])
                nc.sync.dma_start(out=dbg_hf[i * P:(i + 1) * P, :], in_=hTf[i])
                nc.sync.dma_start(out=dbg_q[i * P:(i + 1) * P, :], in_=qT[i])
                nc.sync.dma_start(out=dbg_k[i * P:(i + 1) * P, :], in_=kT[i])
            nc.sync.dma_start(out=dbg_v[:, :], in_=vG)

        # ---- Phase B: attention per head-pair group ----
        sM = ExitStack()
        attnp = sM.enter_context(tc.tile_pool(name="attnp", bufs=1, side="right"))
        attnT = _alloc(attnp, FT_C, [P, TOWN], BF16, "attnT")
        pw = _alloc(attnp, KT_C, [P, C], BF16, "pw")
        xo32 = _alloc(attnp, FT_C, [P, TOWN], F32, "xo32")

        with ExitStack() as cB:
            mpool = cB.enter_context(tc.tile_pool(name="mpool", bufs=1))
            m0 = _alloc(mpool, 4, [P, 1024], BF16, "m0")
            m1 = _alloc(mpool, 4, [P, 1024], BF16, "m1")
            for k2 in range(4):
                nc.sync.dma_start(out=m0[k2], in_=mask0[k2 * P:(k2 + 1) * P, :])
                nc.sync.dma_start(out=m1[k2], in_=mask1[k2 * P:(k2 + 1) * P, :])
            for kt in range(KT_C):
                nc.sync.dma_start(out=pw[kt],
                                  in_=proj_w[kt * P:(kt + 1) * P, :])
                nc.sync.dma_start(out=xo32[kt],
                                  in_=xTo_f32[kt * P:(kt + 1) * P, :])

            sc_ps = cB.enter_context(
                tc.tile_pool(name="sc_ps", bufs=2, space="PSUM"))
            y_ps_pool = cB.enter_context(
                tc.tile_pool(name="y_ps_pool", bufs=1, space="PSUM"))
            ppool = cB.enter_context(tc.tile_pool(name="ppool", bufs=3))
            npool = cB.enter_context(tc.tile_pool(name="npool", bufs=2))

            for g in range(NGROUP):
                y_ps = {
                    (qc, hh): y_ps_pool.tile([65, 512], F32, tag=f"y{qc}{hh}",
                                             name=f"y{g}_{qc}_{hh}")
                    for qc in range(2) for hh in range(2)
                }
                for k2 in range(8):
                    for hh in range(2):
                        hsl = slice(64 * hh, 64 * (hh + 1))
                        scs = {}
                        if k2 < 4:
                            scs[0] = sc_ps.tile([P, 1024], F32, tag="sc",
                                                name=f"sc{g}_0_{k2}_{hh}")
                        scs[1] = sc_ps.tile([P, 1024], F32, tag="sc",
                                            name=f"sc{g}_1_{k2}_{hh}")
                        for j in range(2):
                            kt = 2 * k2 + j
                            ksl = kT[g][hsl, kt * P:(kt + 1) * P]
                            for qc in scs:
                                nc.tensor.matmul(
                                    scs[qc][:, j * 512:(j + 1) * 512],
                                    ksl,
                                    qT[g][hsl, qc * 512:(qc + 1) * 512],
                                    start=True, stop=True,
                                    tile_position=(64 * hh, 0))
                        pts = {}
                        for qc in scs:
                            pts[qc] = ppool.tile([P, 1024], BF16, tag="pt",
                                                 name=f"p{g}_{qc}_{k2}_{hh}")
                            nc.scalar.activation(out=pts[qc], in_=scs[qc],
                                                 func=Exp, scale=SCALE)
                        if k2 < 4:
                            nc.vector.tensor_mul(out=pts[0], in0=pts[0],
                                                 in1=m0[k2])
                        else:
                            nc.vector.tensor_mul(out=pts[1], in0=pts[1],
                                                 in1=m1[k2 - 4])
                        if DEBUG and g == 0 and hh == 0:
                            qcd = 0 if k2 < 4 else 1
                            idx = 8 * qcd + (k2 % 4) * 2
                            nc.sync.dma_start(
                                out=dbg_pt[idx * P:(idx + 1) * P, :],
                                in_=pts[qcd])
                        for j in range(2):
                            kt = 2 * k2 + j
                            vsl = vG4[:, kt, 2 * g + hh, :]
                            for qc in pts:
                                nc.tensor.matmul(
                                    y_ps[(qc, hh)],
                                    vsl,
                                    pts[qc][:, j * 512:(j + 1) * 512],
                                    start=(kt == 0),
                                    stop=(kt == (7 if qc == 0 else 15)))
                for qc in range(2):
                    for hh in range(2):
                        dn = npool.tile([1, 512], F32, tag="dn",
                                        name=f"dn{g}_{qc}_{hh}")
                        nc.vector.tensor_copy(out=dn,
                                              in_=y_ps[(qc, hh)][64:65, :])
                        r = npool.tile([1, 512], F32, tag="r",
                                       name=f"r{g}_{qc}_{hh}")
                        nc.vector.reciprocal_approx_fast(out=r, in_=dn)
                        if DEBUG:
                            nc.sync.dma_start(
                                out=dbg_r[g * 4 + qc * 2 + hh:
                                          g * 4 + qc * 2 + hh + 1, :],
                                in_=r)
                        rb = npool.tile([64, 512], F32, tag="rb",
                                        name=f"rb{g}_{qc}_{hh}")
                        nc.gpsimd.partition_broadcast(rb, r[0:1, :])
                        nc.vector.tensor_mul(
                            out=attnT[g][64 * hh:64 * (hh + 1),
                                         qc * 512:(qc + 1) * 512],
                            in0=y_ps[(qc, hh)][0:64, :], in1=rb)

        if DEBUG:
            for i in range(FT_C):
                nc.sync.dma_start(out=dbg_at[i * P:(i + 1) * P, :], in_=attnT[i])
        s1.close()   # free qT/kT/vG

        # ---- proj + residual -> x2T; LN2 -> h2T ----
        x2p = top.enter_context(tc.tile_pool(name="x2p", bufs=1))
        x2T = _alloc(x2p, FT_C, [P, TOWN], F32R, "x2T")
        h2T = _alloc(x2p, FT_C, [P, TOWN], BF16, "h2T")
        if True:
            with ExitStack() as cC:
                pj_ps = cC.enter_context(
                    tc.tile_pool(name="pj_ps", bufs=2, space="PSUM"))
                st2 = cC.enter_context(
                    tc.tile_pool(name="st2", bufs=2, space="PSUM"))
                rowp2 = cC.enter_context(tc.tile_pool(name="rowp2", bufs=1))
                bcp2 = cC.enter_context(tc.tile_pool(name="bcp2", bufs=2))
                apl2 = cC.enter_context(tc.tile_pool(name="apl2", bufs=1))

                for ft in range(FT_C):
                    pps = [pj_ps.tile([P, 512], F32, tag=f"pj{nb}",
                                      name=f"pps{ft}_{nb}") for nb in range(2)]
                    for kt in range(KT_C):
                        for nb in range(2):
                            nc.tensor.matmul(
                                pps[nb], pw[kt][:, ft * P:(ft + 1) * P],
                                attnT[kt][:, nb * 512:(nb + 1) * 512],
                                start=(kt == 0), stop=(kt == KT_C - 1))
                    for nb in range(2):
                        sl2 = slice(nb * 512, (nb + 1) * 512)
                        nc.vector.scalar_tensor_tensor(
                            out=x2T[ft][:, sl2], in0=pps[nb],
                            scalar=projb_t[:, ft:ft + 1],
                            in1=xo32[ft][:, sl2], op0=ADD, op1=ADD)

                # LN2 in f32 (x2T is the precise residual stream)
                for nb in range(2):
                    sl = slice(nb * 512, (nb + 1) * 512)
                    ssum = st2.tile([1, 512], F32, tag="ss2", name=f"l2ss{nb}")
                    ssq = st2.tile([1, 512], F32, tag="sq2", name=f"l2sq{nb}")
                    for kt in range(KT_C):
                        nc.tensor.matmul(ssum, ones_r, x2T[kt][:, sl],
                                         start=(kt == 0), stop=(kt == KT_C - 1))
                    for kt in range(KT_C):
                        sq = rowp2.tile([P, 512], F32R, tag=f"sqt{kt}",
                                        name=f"l2sqt{nb}_{kt}", bufs=1)
                        nc.vector.tensor_mul(out=sq, in0=x2T[kt][:, sl],
                                             in1=x2T[kt][:, sl])
                        nc.tensor.matmul(ssq, ones_r, sq,
                                         start=(kt == 0), stop=(kt == KT_C - 1))
                    mu = rowp2.tile([1, 512], F32, tag="mu", name=f"l2mu{nb}")
                    nc.scalar.mul(mu, ssum[0:1, :], 1.0 / C)
                    msq = rowp2.tile([1, 512], F32, tag="msq", name=f"l2msq{nb}")
                    nc.scalar.mul(msq, ssq[0:1, :], 1.0 / C)
                    var = rowp2.tile([1, 512], F32, tag="var", name=f"l2var{nb}")
                    nc.vector.tensor_mul(out=var, in0=mu, in1=mu)
                    nc.vector.tensor_sub(out=var, in0=msq, in1=var)
                    std = rowp2.tile([1, 512], F32, tag="std", name=f"l2std{nb}")
                    nc.scalar.activation(out=std, in_=var, func=Sqrt,
                                         bias=eps_t[0:1, 0:1], scale=1.0)
                    rs = rowp2.tile([1, 512], F32, tag="rs", name=f"l2rs{nb}")
                    nc.vector.reciprocal_approx_fast(out=rs, in_=std)
                    mu_b = bcp2.tile([P, 512], F32, tag="mub", name=f"l2mub{nb}")
                    nc.gpsimd.partition_broadcast(mu_b, mu)
                    rs_b = bcp2.tile([P, 512], F32, tag="rsb", name=f"l2rsb{nb}")
                    nc.gpsimd.partition_broadcast(rs_b, rs)
                    for ft in range(FT_C):
                        t = apl2.tile([P, 512], F32, tag=f"ap{ft}",
                                      name=f"l2ap{nb}_{ft}", bufs=1)
                        nc.vector.tensor_sub(out=t,
                                             in0=x2T[ft][:, sl].bitcast(F32),
                                             in1=mu_b)
                        nc.vector.tensor_mul(out=t, in0=t, in1=rs_b)
                        nc.scalar.activation(out=h2T[ft][:, sl], in_=t,
                                             func=Ident,
                                             bias=ln2b_t[:, ft:ft + 1],
                                             scale=ln2g_t[:, ft:ft + 1])

        if DEBUG:
            for i in range(FT_C):
                nc.sync.dma_start(out=dbg_x2[i * P:(i + 1) * P, :],
                                  in_=x2T[i].bitcast(F32))
                nc.sync.dma_start(out=dbg_h2[i * P:(i + 1) * P, :], in_=h2T[i])
        sM.close()   # free attnT/pw/xo32

        # ---- Phase C: MLP ----
        with ExitStack() as cD:
            h1p = cD.enter_context(tc.tile_pool(name="h1p", bufs=1))
            h1 = _alloc(h1p, DFF // P, [P, TOWN], BF16, "h1")
            w1p = cD.enter_context(tc.tile_pool(name="w1p", bufs=1))
            with ExitStack() as cD1:
                f1_ps = cD1.enter_context(
                    tc.tile_pool(name="f1_ps", bufs=2, space="PSUM"))
                for dh in range(2):
                    w1h = [w1p.tile([P, DFF // 2], BF16, tag=f"w1_{kt}",
                                    name=f"w1h{dh}_{kt}", bufs=2)
                           for kt in range(KT_C)]
                    for kt in range(KT_C):
                        nc.sync.dma_start(
                            out=w1h[kt],
                            in_=fc1_w[kt * P:(kt + 1) * P,
                                      dh * (DFF // 2):(dh + 1) * (DFF // 2)])
                    for dtl in range(16):
                        dt = dh * 16 + dtl
                        fps = [f1_ps.tile([P, 512], F32, tag=f"f1{nb}",
                                          name=f"fps{dt}_{nb}")
                               for nb in range(2)]
                        for kt in range(KT_C):
                            for nb in range(2):
                                nc.tensor.matmul(
                                    fps[nb], w1h[kt][:, dtl * P:(dtl + 1) * P],
                                    h2T[kt][:, nb * 512:(nb + 1) * 512],
                                    start=(kt == 0), stop=(kt == KT_C - 1))
                        for nb in range(2):
                            nc.scalar.activation(
                                out=h1[dt][:, nb * 512:(nb + 1) * 512],
                                in_=fps[nb], func=Relu,
                                bias=fc1b_t[:, dt:dt + 1], scale=1.0)

            with ExitStack() as cD2:
                f2_ps = cD2.enter_context(
                    tc.tile_pool(name="f2_ps", bufs=1, space="PSUM"))
                w2p = cD2.enter_context(tc.tile_pool(name="w2p", bufs=1))
                opool = cD2.enter_context(tc.tile_pool(name="opool", bufs=3))
                for fb in range(2):
                    fp2 = [f2_ps.tile([P, TOWN], F32, tag=f"f2_{i}",
                                      name=f"fp2_{fb}_{i}")
                           for i in range(4)]
                    for k8 in range(DFF // P):
                        w2t = w2p.tile([P, C], BF16, tag="w2",
                                       name=f"w2_{fb}_{k8}", bufs=6)
                        nc.sync.dma_start(
                            out=w2t, in_=fc2_w[k8 * P:(k8 + 1) * P, :])
                        for i in range(4):
                            ft = fb * 4 + i
                            for nb in range(2):
                                nc.tensor.matmul(
                                    fp2[i][:, nb * 512:(nb + 1) * 512],
                                    w2t[:, ft * P:(ft + 1) * P],
                                    h1[k8][:, nb * 512:(nb + 1) * 512],
                                    start=(k8 == 0), stop=(k8 == DFF // P - 1))
                    for i in range(4):
                        ft = fb * 4 + i
                        o = opool.tile([P, TOWN], F32, tag="o", name=f"o{ft}")
                        nc.vector.scalar_tensor_tensor(
                            out=o, in0=fp2[i], scalar=fc2b_t[:, ft:ft + 1],
                            in1=x2T[ft].bitcast(F32), op0=ADD, op1=ADD)
                        nc.sync.dma_start(out=out[ft * P:(ft + 1) * P, :],
                                          in_=o)

    nc.compile()
    return nc


_NC_CACHE = None


def _get_nc():
    global _NC_CACHE
    if _NC_CACHE is None:
        _NC_CACHE = build_nc()
    return _NC_CACHE


_CHUNKS = {0: (0, 3), 1: (1, 2)}


def _pair_mask(m):
    # [1024, 512] -> [512, 1024]: row-block k2 holds [mask(2*k2) | mask(2*k2+1)]
    return np.ascontiguousarray(
        m.reshape(4, 2, 128, 512).transpose(0, 2, 1, 3).reshape(512, 1024))


def _make_masks(cl, ch):
    # multiplicative {0,1} masks, applied to probabilities after exp
    k = np.arange(1024, dtype=np.int64)[:, None]
    q = np.arange(512, dtype=np.int64)[None, :]
    m_lo = (k <= cl * 512 + q).astype(np.float32)
    m_hi = (1024 + k <= ch * 512 + q).astype(np.float32)
    return _pair_mask(m_lo), _pair_mask(m_hi)


def _run(inputs, trace=False):
    nc = _get_nc()
    bf = ml_dtypes.bfloat16
    xs = {k: np.ascontiguousarray(np.asarray(v), dtype=np.float32)
          for k, v in inputs.items()}
    x = xs["x"]
    xT = {b: np.ascontiguousarray(x[b].T) for b in range(B)}
    wb = {k: np.ascontiguousarray(xs[k].astype(bf))
          for k in ("attn_w", "proj_w", "fc1_w", "fc2_w")}
    in_maps = []
    for c in range(8):
        b, j = divmod(c, 2)
        cl, ch = _CHUNKS[j]
        m_lo, m_hi = _make_masks(cl, ch)
        xT_own = np.ascontiguousarray(
            np.concatenate([xT[b][:, cl * 512:(cl + 1) * 512],
                            xT[b][:, ch * 512:(ch + 1) * 512]], axis=1))
        in_maps.append({
            "xT_b16": np.ascontiguousarray(xT[b].astype(bf)),
            "xTo_b16": xT_own.astype(bf),
            "xTo_f32": xT_own,
            "mask0": m_lo.astype(bf),
            "mask1": m_hi.astype(bf),
            "attn_w": wb["attn_w"], "attn_b": xs["attn_b"],
            "proj_w": wb["proj_w"], "proj_b": xs["proj_b"],
            "ln1_g": xs["ln1_g"], "ln1_b": xs["ln1_b"],
            "ln2_g": xs["ln2_g"], "ln2_b": xs["ln2_b"],
            "fc1_w": wb["fc1_w"], "fc1_b": xs["fc1_b"],
            "fc2_w": wb["fc2_w"], "fc2_b": xs["fc2_b"],
        })
    res = run_bass_kernel_spmd(nc, in_maps, list(range(8)), trace=trace)
    full = np.empty((B, T, C), dtype=np.float32)
    for c in range(8):
        b, j = divmod(c, 2)
        cl, ch = _CHUNKS[j]
        o = res.results[c]["out"]            # [C, TOWN] feature-major
        full[b, cl * 512:(cl + 1) * 512] = o[:, 0:512].T
        full[b, ch * 512:(ch + 1) * 512] = o[:, 512:1024].T
    return full, res.exec_time_ns


def kernel(**inputs):
    out, _ = _run(inputs, trace=False)
    return out

